# revision 19
# baseline (speedup 1.0000x reference)
"""Trainium2 Bass kernel for nn_DenoisingSharpening (v2).

Contract: kernel(**inputs) takes the FULL unsharded inputs
(images [8,64,64,64,3] f32, params [8,64,7] f32, k [] f32) and returns
the FULL output [8,64,64,64,3] f32.

Strategy
--------
Data-parallel over N = B*P = 512 images; 64 images per NeuronCore, one
half-image (32 rows) per SBUF partition -> 128 partitions x 8 cores.

Host ships channel-planar float16 padded halves [3, 34, 66] so every
device op keeps a packed (stride-1) last dim => DVE 2x f16 throughput,
and channel broadcasts ride on outer AP dims (no slow broadcast ops).

Device (per chunk of CR rows):
  * 4 unique bilateral tap fields (W, NW, N, NE) on uniform [SR,3,66]
    row-major boxes; NE stored pre-shifted one column so all four taps
    share identical interior windows.
  * ck = exp(-(s*d)^2 sum + log w) with squares on ACT, channel adds on
    DVE, exp batched per weight class (edge/corner) on ACT.
  * numerator via the symmetric-pair trick: dif = prod_I - prod_M,
    denominator from ck window adds (Pool engine).
  * bf - x = nacc / sacc with native DVE reciprocal (f32), f16 rescale.
  * separable gaussian detail rebuilt from the SAME dW/dN diff fields.
  * noise chain: abs/tanh/square/exp on ACT (one table set), the
    division with the DVE divide ALU, min+accum via tensor_scalar.
  * skip decision moved to host: device ships per-chunk sums of
    |detail| and clip(noise)/tau; host swaps skipped images exactly.
"""

import numpy as np

N_CORES = 8
B, PP, H, W, C = 8, 64, 64, 64, 3
NIMG = B * PP          # 512
HALVES = 2 * NIMG      # 1024 half-images, 128 per core
PR, PC = 34, 66        # padded half-image rows/cols
ROWS_PER_HALF = 32

NOISE_THRESH = 0.002
SKIP_THRESH = 1e-4
MEAN_N = float(C * H * W)

# params columns
(P_S, P_LOGE, P_LOGC, P_WSC, P_BE, P_1M2BE, P_IGT, P_OFFGT, P_CLIP,
 P_KT, P_KTB, P_SQL2, P_NSQL2) = range(13)
NPARAM = 16

_CACHE = {}


# --------------------------------------------------------------------------
# host-side preprocessing
# --------------------------------------------------------------------------

def _host_prep(images, params, k):
    x = np.ascontiguousarray(images, dtype=np.float32).reshape(NIMG, H, W, C)
    xp = np.pad(x, ((0, 0), (1, 1), (1, 1), (0, 0)), mode="reflect")
    halves = np.stack([xp[:, 0:PR], xp[:, ROWS_PER_HALF:ROWS_PER_HALF + PR]],
                      axis=1).reshape(HALVES, PR, PC, C)
    planar = np.ascontiguousarray(
        halves.transpose(0, 3, 1, 2), dtype=np.float16)  # [1024, 3, 34, 66]

    p = np.asarray(params, dtype=np.float32).reshape(NIMG, 7)
    sigma_r = np.clip(p[:, 1], 0.01, 1.0)
    sigma_s = np.clip(p[:, 0], 0.2, 5.0)
    sigma_f = np.clip(p[:, 2], 0.2, 3.0)
    lam = np.clip(p[:, 3], 0.1, 2.0)
    tau = np.clip(p[:, 4], 0.5, 5.0)
    gain = np.clip(p[:, 5], 0.2, 2.0)
    offset = np.clip(p[:, 6], 0.01, 1.0)

    def gauss1d(sig):
        g = np.exp(-0.5 * (np.array([-1.0, 0.0, 1.0], np.float32)[None, :]
                           / sig[:, None]) ** 2)
        return g / g.sum(axis=1, keepdims=True)

    gs = gauss1d(sigma_s)
    gf = gauss1d(sigma_f)
    aE, aC = gs[:, 0], gs[:, 1]
    bE = gf[:, 0]

    kpos = max(abs(float(np.asarray(k))), 1.0)
    gt = gain / tau
    sql2 = np.sqrt(lam * bE)

    pars = np.zeros((NIMG, NPARAM), np.float32)
    pars[:, P_S] = np.sqrt(0.5) / sigma_r
    pars[:, P_LOGE] = np.log(aE * aC)
    pars[:, P_LOGC] = np.log(aE * aE)
    pars[:, P_WSC] = aC * aC
    pars[:, P_BE] = bE
    pars[:, P_1M2BE] = 1.0 - 2.0 * bE
    pars[:, P_IGT] = 1.0 / gt
    pars[:, P_OFFGT] = offset / gt
    pars[:, P_CLIP] = 10.0 / tau
    pars[:, P_KT] = 0.5 * kpos
    pars[:, P_KTB] = -0.5 * kpos * NOISE_THRESH
    pars[:, P_SQL2] = sql2
    pars[:, P_NSQL2] = -sql2
    pars2 = np.repeat(pars, 2, axis=0)  # per half-image

    in_maps = []
    per_core = HALVES // N_CORES
    for c in range(N_CORES):
        sl = slice(c * per_core, (c + 1) * per_core)
        in_maps.append({
            "xpad": np.ascontiguousarray(planar[sl]),
            "pp": np.ascontiguousarray(pars2[sl]),
        })
    return in_maps


def _host_post(results, images, params, chunks):
    cr = ROWS_PER_HALF // chunks
    outs = [np.asarray(r["out"], np.float32) for r in results]
    full = np.concatenate(outs, axis=0)  # [1024, chunks, cr, 3, 64]
    full = full.reshape(HALVES, ROWS_PER_HALF, C, W).transpose(0, 1, 3, 2)
    full = full.reshape(NIMG, H, W, C)

    sk = np.concatenate([np.asarray(r["accs"], np.float64)
                         for r in results], axis=0)  # [1024, 2*chunks]
    a_half = sk[:, 0:chunks].sum(axis=1)
    n_half = sk[:, chunks:2 * chunks].sum(axis=1)
    a_img = a_half[0::2] + a_half[1::2]
    n_img = n_half[0::2] + n_half[1::2]
    tau = np.clip(np.asarray(params, np.float32).reshape(NIMG, 7)[:, 4],
                  0.5, 5.0)
    skip = (a_img < MEAN_N * SKIP_THRESH) | (n_img < MEAN_N * SKIP_THRESH / tau)
    if skip.any():
        x = np.asarray(images, np.float32).reshape(NIMG, H, W, C)
        full[skip] = np.clip(x[skip], 1e-5, 1.0)
    return full.reshape(B, PP, H, W, C)


# --------------------------------------------------------------------------
# device program
# --------------------------------------------------------------------------

def build_program(cfg=None):
    import concourse.tile as tile
    from concourse import bacc, mybir
    from contextlib import ExitStack

    cfg = dict(cfg or {})
    F32 = mybir.dt.float32
    F16 = mybir.dt.float16
    ALU = mybir.AluOpType
    AF = mybir.ActivationFunctionType

    repeat = int(cfg.get("repeat", 1))
    CHUNKS = int(cfg.get("chunks", 4))
    CR = ROWS_PER_HALF // CHUNKS
    SR = CR + 2
    # engine knobs
    pool_pair = bool(cfg.get("pool_pair", True))
    pool_d2 = int(cfg.get("pool_d2", 0))      # taps whose d2 adds go to Pool
    pool_ptree = bool(cfg.get("pool_ptree", True))
    pool_av = bool(cfg.get("pool_av", True))
    pool_c3 = bool(cfg.get("pool_c3", False))
    div_eng = cfg.get("div", "pool")          # 'dve' | 'pool'
    act_ts = bool(cfg.get("act_ts", True))    # d1/th1/c1 on ACT

    nc = bacc.Bacc("TRN2", target_bir_lowering=False, debug=False)
    xdram = nc.dram_tensor("xpad", [128, C, PR, PC], F16,
                           kind="ExternalInput").ap()
    pdram = nc.dram_tensor("pp", [128, NPARAM], F32,
                           kind="ExternalInput").ap()
    odram = nc.dram_tensor("out", [128, CHUNKS, CR, C, W], F16,
                           kind="ExternalOutput").ap()
    adram = nc.dram_tensor("accs", [128, 2 * CHUNKS], F32,
                           kind="ExternalOutput").ap()

    with tile.TileContext(nc) as tc:
        with ExitStack() as ctx:
            pool = ctx.enter_context(tc.tile_pool(name="main", bufs=1))

            pp = pool.tile([128, NPARAM], F32, tag="pp", bufs=1)
            nc.sync.dma_start(pp[:], pdram[:])

            def par(col):
                return pp[:, col:col + 1]

            for rep in range(repeat):
              xs = pool.tile([128, C, PR, PC], F16, tag="xs", bufs=2,
                             name=f"xs{rep}")
              nc.sync.dma_start(xs[:], xdram[:])
              accs = pool.tile([128, 2 * CHUNKS], F32, tag="accs", bufs=2,
                               name=f"accs{rep}")

              for ch in range(CHUNKS):
                R = ch * CR
                sfx = f"{ch}_{rep}"

                def xv(r0, r1, c0, c1):
                    # planar view rearranged to (row, ch, col) iteration
                    return xs[:, :, R + r0:R + r1, c0:c1].rearrange(
                        "p c r j -> p r c j")

                # ---- tap diff fields, uniform [SR,3,66] boxes ----
                dW = pool.tile([128, SR, C, PC], F16, tag="dWN", bufs=4,
                               name=f"dW{sfx}")
                nc.vector.tensor_tensor(
                    dW[:, 0:SR, :, 1:66], xv(0, SR, 0, 65), xv(0, SR, 1, 66),
                    ALU.subtract)
                dN = pool.tile([128, SR, C, PC], F16, tag="dWN", bufs=4,
                               name=f"dN{sfx}")
                nc.vector.tensor_tensor(
                    dN[:, 1:SR, :, 1:66], xv(0, SR - 1, 1, 66),
                    xv(1, SR, 1, 66), ALU.subtract)
                dNW = pool.tile([128, SR, C, PC], F16, tag="dC", bufs=4,
                                name=f"dNW{sfx}")
                nc.vector.tensor_tensor(
                    dNW[:, 1:SR, :, 1:66], xv(0, SR - 1, 0, 65),
                    xv(1, SR, 1, 66), ALU.subtract)
                # NE stored shifted right one col: tile col j = padded col j-1
                dNE = pool.tile([128, SR, C, PC], F16, tag="dC", bufs=4,
                                name=f"dNE{sfx}")
                nc.vector.tensor_tensor(
                    dNE[:, 1:SR, :, 1:66], xv(0, SR - 1, 1, 66),
                    xv(1, SR, 0, 65), ALU.subtract)

                # ---- squares (ACT), channel sums -> d2, exp -> ck ----
                d2EN = pool.tile([128, 2, SR, PC], F16, tag="d2", bufs=4,
                                 name=f"d2EN{sfx}")
                d2C = pool.tile([128, 2, SR, PC], F16, tag="d2", bufs=4,
                                name=f"d2C{sfx}")
                for ti, (dt_, d2t, g) in enumerate(
                        [(dW, d2EN, 0), (dN, d2EN, 1),
                         (dNW, d2C, 0), (dNE, d2C, 1)]):
                    sq = pool.tile([128, SR, C, PC], F16, tag="sq", bufs=3,
                                   name=f"sq{ti}_{sfx}")
                    nc.scalar.activation(
                        sq[:, 1:SR, :, 1:66], dt_[:, 1:SR, :, 1:66],
                        AF.Square, scale=par(P_S))
                    eng = nc.gpsimd if ti < pool_d2 else nc.vector
                    d2a = pool.tile([128, SR, PC], F16, tag="d2a", bufs=2,
                                    name=f"d2a{ti}_{sfx}")
                    eng.tensor_tensor(
                        d2a[:, 1:SR, 1:66], sq[:, 1:SR, 0, 1:66],
                        sq[:, 1:SR, 1, 1:66], ALU.add)
                    eng.tensor_tensor(
                        d2t[:, g, 1:SR, 1:66], d2a[:, 1:SR, 1:66],
                        sq[:, 1:SR, 2, 1:66], ALU.add)

                ckEN = pool.tile([128, 2, SR, PC], F16, tag="ck", bufs=4,
                                 name=f"ckEN{sfx}")
                nc.scalar.activation(
                    ckEN[:, :, 1:SR, 1:66], d2EN[:, :, 1:SR, 1:66],
                    AF.Exp, bias=par(P_LOGE), scale=-1.0)
                ckC = pool.tile([128, 2, SR, PC], F16, tag="ck", bufs=4,
                                name=f"ckC{sfx}")
                nc.scalar.activation(
                    ckC[:, :, 1:SR, 1:66], d2C[:, :, 1:SR, 1:66],
                    AF.Exp, bias=par(P_LOGC), scale=-1.0)

                # ---- prod, dif (I - M windows), pair ----
                # windows in tile coords: I = rows 1..CR, cols 1..64 for all
                # taps except NE whose I is cols 2..65 (shifted storage).
                TAPW = [(dW, ckEN, 0, (1, 1 + CR, 2, 66)),    # M: same rows, col+1
                        (dN, ckEN, 1, (2, SR, 1, 65)),        # M: row+1
                        (dNW, ckC, 0, (2, SR, 2, 66)),        # M: row+1, col+1
                        (dNE, ckC, 1, (2, SR, 1, 65))]        # shifted: I col+1
                difs = []
                pairs = []
                for ti, (dt_, ckt, g, (mr0, mr1, mc0, mc1)) in enumerate(TAPW):
                    prod = pool.tile([128, SR, C, PC], F16, tag="prod", bufs=4,
                                     name=f"pr{ti}_{sfx}")
                    nc.vector.tensor_tensor(
                        prod[:, 1:SR, :, 1:66], dt_[:, 1:SR, :, 1:66],
                        ckt[:, g, 1:SR, 1:66].unsqueeze(2).broadcast_to(
                            [128, SR - 1, C, 65]), ALU.mult)
                    i_c0 = 2 if ti == 3 else 1
                    dif = pool.tile([128, CR, C, W], F16, tag="dif", bufs=3,
                                    name=f"dif{ti}_{sfx}")
                    nc.vector.tensor_tensor(
                        dif[:], prod[:, 1:1 + CR, :, i_c0:i_c0 + W],
                        prod[:, mr0:mr0 + CR, :, mc0:mc0 + W], ALU.subtract)
                    difs.append(dif)
                    pair = pool.tile([128, CR, W], F16, tag="pair", bufs=3,
                                     name=f"pai{ti}_{sfx}")
                    peng = nc.gpsimd if pool_pair else nc.vector
                    peng.tensor_tensor(
                        pair[:], ckt[:, g, 1:1 + CR, i_c0:i_c0 + W],
                        ckt[:, g, mr0:mr0 + CR, mc0:mc0 + W], ALU.add)
                    pairs.append(pair)

                n01 = pool.tile([128, CR, C, W], F16, tag="nacc", bufs=3,
                                name=f"n01{sfx}")
                nc.vector.tensor_tensor(n01[:], difs[0][:], difs[1][:], ALU.add)
                n23 = pool.tile([128, CR, C, W], F16, tag="nacc", bufs=3,
                                name=f"n23{sfx}")
                nc.vector.tensor_tensor(n23[:], difs[2][:], difs[3][:], ALU.add)
                nacc = pool.tile([128, CR, C, W], F16, tag="nacc", bufs=3,
                                 name=f"nacc{sfx}")
                nc.vector.tensor_tensor(nacc[:], n01[:], n23[:], ALU.add)

                teng = nc.gpsimd if pool_ptree else nc.vector
                p01 = pool.tile([128, CR, W], F16, tag="ptree", bufs=3,
                                name=f"p01{sfx}")
                teng.tensor_tensor(p01[:], pairs[0][:], pairs[1][:], ALU.add)
                p23 = pool.tile([128, CR, W], F16, tag="ptree", bufs=3,
                                name=f"p23{sfx}")
                teng.tensor_tensor(p23[:], pairs[2][:], pairs[3][:], ALU.add)
                q = pool.tile([128, CR, W], F16, tag="ptree", bufs=3,
                              name=f"q{sfx}")
                teng.tensor_tensor(q[:], p01[:], p23[:], ALU.add)
                sacc = pool.tile([128, CR, W], F32, tag="sacc", bufs=3,
                                 name=f"sacc{sfx}")
                nc.vector.tensor_scalar(sacc[:], q[:], par(P_WSC), None,
                                        ALU.add)
                rS = pool.tile([128, CR, W], F32, tag="rS", bufs=3,
                               name=f"rS{sfx}")
                nc.vector.reciprocal_approx_fast(
                    rS[:].rearrange("p a b -> p (a b)"),
                    sacc[:].rearrange("p a b -> p (a b)"))
                rs16 = pool.tile([128, CR, W], F16, tag="rs16", bufs=3,
                                 name=f"rs16{sfx}")
                nc.scalar.copy(rs16[:], rS[:])
                tT = pool.tile([128, CR, C, W], F16, tag="tT", bufs=2,
                               name=f"tT{sfx}")
                nc.vector.tensor_tensor(
                    tT[:], nacc[:],
                    rs16[:].unsqueeze(2).broadcast_to([128, CR, C, W]),
                    ALU.mult)

                # ---- gaussian detail from dW/dN ----
                Hf = pool.tile([128, SR, C, W], F16, tag="H", bufs=2,
                               name=f"H{sfx}")
                nc.vector.tensor_tensor(
                    Hf[:], dW[:, :, :, 2:66], dW[:, :, :, 1:65], ALU.subtract)
                V = pool.tile([128, CR, C, W], F16, tag="s1", bufs=5,
                              name=f"V{sfx}")
                nc.vector.tensor_tensor(
                    V[:], dN[:, 2:SR, :, 1:65], dN[:, 1:1 + CR, :, 1:65],
                    ALU.subtract)
                av = pool.tile([128, CR, C, W], F16, tag="s1", bufs=5,
                               name=f"av{sfx}")
                (nc.gpsimd if pool_av else nc.vector).tensor_tensor(
                    av[:], Hf[:, 0:CR], Hf[:, 2:SR], ALU.add)
                c1 = pool.tile([128, CR, C, W], F16, tag="s1", bufs=5,
                               name=f"c1{sfx}")
                c2 = pool.tile([128, CR, C, W], F16, tag="s1", bufs=5,
                               name=f"c2{sfx}")
                if act_ts:
                    nc.scalar.activation(c1[:], Hf[:, 1:1 + CR], AF.Copy,
                                         scale=par(P_1M2BE))
                else:
                    nc.vector.tensor_scalar(c1[:], Hf[:, 1:1 + CR],
                                            par(P_1M2BE), None, ALU.mult)
                nc.vector.tensor_scalar(c2[:], av[:], par(P_BE), None,
                                        ALU.mult)
                c3 = pool.tile([128, CR, C, W], F16, tag="s1", bufs=5,
                               name=f"c3{sfx}")
                (nc.gpsimd if pool_c3 else nc.vector).tensor_tensor(
                    c3[:], V[:], c2[:], ALU.add)
                inner = pool.tile([128, CR, C, W], F16, tag="inner", bufs=2,
                                  name=f"inner{sfx}")
                nc.vector.tensor_tensor(inner[:], c1[:], c3[:], ALU.add)

                # ---- noise / masks ----
                adet = pool.tile([128, CR, C, W], F16, tag="s1", bufs=5,
                                 name=f"adet{sfx}")
                nc.scalar.activation(adet[:], inner[:], AF.Abs,
                                     scale=par(P_BE),
                                     accum_out=accs[:, ch:ch + 1])
                th = pool.tile([128, CR, C, W], F16, tag="s1", bufs=5,
                               name=f"th{sfx}")
                nc.scalar.activation(th[:], adet[:], AF.Tanh,
                                     bias=par(P_KTB), scale=par(P_KT))
                th1 = pool.tile([128, CR, C, W], F16, tag="th1", bufs=2,
                                name=f"th1{sfx}")
                if act_ts:
                    nc.scalar.activation(th1[:], th[:], AF.Copy,
                                         scale=0.5, bias=0.5)
                else:
                    nc.vector.tensor_scalar(th1[:], th[:], 0.5, 0.5,
                                            ALU.mult, ALU.add)
                d1 = pool.tile([128, CR, C, W], F32, tag="d32", bufs=2,
                               name=f"d1{sfx}")
                nc.scalar.activation(d1[:], xv(1, 1 + CR, 1, 65), AF.Identity,
                                     scale=par(P_IGT), bias=par(P_OFFGT))
                r1 = pool.tile([128, CR, C, W], F32, tag="d32", bufs=2,
                               name=f"r1{sfx}")
                nc.vector.reciprocal_approx_fast(
                    r1[:].rearrange("p a b c -> p (a b c)"),
                    d1[:].rearrange("p a b c -> p (a b c)"))
                ne0 = pool.tile([128, CR, C, W], F16, tag="s1", bufs=5,
                                name=f"ne0{sfx}")
                deng = nc.gpsimd if div_eng == "pool" else nc.vector
                deng.tensor_tensor(ne0[:], adet[:], r1[:], ALU.mult)
                neq = pool.tile([128, CR, C, W], F16, tag="s1", bufs=5,
                                name=f"neq{sfx}")
                nc.vector.tensor_scalar(
                    neq[:], ne0[:], par(P_CLIP), None, ALU.min, ALU.add,
                    accum_out=accs[:, CHUNKS + ch:CHUNKS + ch + 1])
                sqn = pool.tile([128, CR, C, W], F16, tag="s1", bufs=5,
                                name=f"sqn{sfx}")
                nc.scalar.activation(sqn[:], neq[:], AF.Square)
                ee = pool.tile([128, CR, C, W], F16, tag="s1", bufs=5,
                               name=f"ee{sfx}")
                nc.scalar.activation(ee[:], sqn[:], AF.Exp, scale=-1.0)
                t2 = pool.tile([128, CR, C, W], F16, tag="s1", bufs=5,
                               name=f"t2{sfx}")
                nc.scalar.activation(t2[:], ee[:], AF.Identity,
                                     scale=par(P_NSQL2), bias=par(P_SQL2))
                nm = pool.tile([128, CR, C, W], F16, tag="s1", bufs=5,
                               name=f"nm{sfx}")
                nc.scalar.activation(nm[:], t2[:], AF.Square)
                s3 = pool.tile([128, CR, C, W], F16, tag="s1", bufs=5,
                               name=f"s3{sfx}")
                nc.vector.tensor_tensor(s3[:], th1[:], nm[:], ALU.mult)
                sharp = pool.tile([128, CR, C, W], F16, tag="s1", bufs=5,
                                  name=f"sh{sfx}")
                nc.vector.tensor_tensor(sharp[:], s3[:], inner[:], ALU.mult)

                # ---- combine + clip + store ----
                t3 = pool.tile([128, CR, C, W], F16, tag="s1", bufs=5,
                               name=f"t3{sfx}")
                nc.vector.tensor_tensor(t3[:], tT[:], sharp[:], ALU.add)
                o3a = pool.tile([128, CR, C, W], F16, tag="s1", bufs=5,
                                name=f"o3a{sfx}")
                nc.vector.tensor_tensor(o3a[:], xv(1, 1 + CR, 1, 65), t3[:],
                                        ALU.add)
                o3 = pool.tile([128, CR, C, W], F16, tag="o3", bufs=2,
                               name=f"o3{sfx}")
                nc.vector.tensor_scalar(o3[:], o3a[:], 1e-5, 1.0,
                                        ALU.max, ALU.min)
                nc.sync.dma_start(odram[:, ch], o3[:])

              nc.sync.dma_start(adram[:], accs[:])

    nc.compile()
    return nc


def _get_program(cfg=None):
    key = tuple(sorted((cfg or {}).items()))
    if key not in _CACHE:
        _CACHE[key] = build_program(cfg)
    return _CACHE[key]


# --------------------------------------------------------------------------
# entry point
# --------------------------------------------------------------------------

DEFAULT_CFG = {}


def kernel(images, params, k):
    from concourse.bass_utils import run_bass_kernel_spmd

    cfg = dict(DEFAULT_CFG)
    nc = _get_program(cfg)
    in_maps = _host_prep(np.asarray(images), np.asarray(params), np.asarray(k))
    res = run_bass_kernel_spmd(nc, in_maps, list(range(N_CORES)))
    chunks = int(cfg.get("chunks", 4))
    return _host_post(res.results, images, params, chunks).astype(np.float32)
